# revision 9
# baseline (speedup 1.0000x reference)
"""ChemConv Bass kernel for 8 TRN2 NeuronCores.

Math: the reference
    node_connection[a,f,i] = sum_n conn[a,n,f] * x[n,i]
    bond_score[a,o,f]      = sum_i node_connection[a,f,i] * pf[o,f,i]
    out[a,o] = sum_f bond_score[a,o,f]*bf[o,f,0] + sum_{f,c} bp[a,f,c]*bf[o,f,1+c]
collapses algebraically to one large matmul plus small ones:
    W[o,f,i]  = pf[o,f,i] * bf[o,f,0]
    Y[k=(f,n), o] = sum_i x[n,i] * W[o,f,i]          (tiny: 24576 x 64)
    out[a,o]  = sum_k conn2d[a,k] * Y[k,o] + sum_j bpT[j,a] * bf2[j,o]
where conn2d[a, (f,n)] = conn[a,n,f] (201 MB -> the memory-bound stream).

Sharding: atoms (dim a) row-slabs of 256 across 8 cores. Each core computes
out_T[o, a_slab] via PSUM accumulation over 192 K-chunks of 128.

conn is pre-packed host-side into a per-core partition-major fp16 slab
[128, KC*256]: row p holds, for every K-chunk kc, the 256 atom columns of
contraction row kc*128+p, laid out consecutively. A batch of B chunks is
then ONE dma_start whose per-partition run is B*512 contiguous bytes on
both the DRAM and SBUF side, so the HWDGE emits ~128 large descriptors
per batch instead of 128*B 1KB ones (descriptor processing, not HBM
bandwidth, limits the naive layout). fp16 halves the stream bytes; the
2^-11 quantization is far inside the 2e-2 gate.

Y is computed on device from x^T (0.5 MB) and cast to fp16 chunk rings.
"""

import numpy as np

import concourse.bass as bass
import concourse.tile as tile
from concourse import bacc, mybir
from concourse.bass_utils import run_bass_kernel_spmd

A = 2048
IN_DEPTH = 64
OUT_DEPTH = 64
F = 12
NCORES = 8
AS = A // NCORES          # 256 atoms per core
K = A * F                 # 24576 contraction length
KP = 128                  # K per matmul chunk (partition dim)
KC = K // KP              # 192 chunks
NBLK = A // KP            # 16 n-blocks per filter tap
KB = 2 * F                # bond-term contraction length (f,c) = 24
YG = 8                    # y chunks per PSUM bank group (8*64 = 512 = bank)

F32 = mybir.dt.float32

_cache = {}


def _conn_dtype(conn_dt):
    if conn_dt == "f16":
        return mybir.dt.float16, np.float16
    if conn_dt == "bf16":
        import ml_dtypes
        return mybir.dt.bfloat16, np.dtype(ml_dtypes.bfloat16)
    return mybir.dt.float32r, np.float32


def _build_nc(repeat=1, B=32, bufs=9, y_ring=24, conn_dt="f16", pre_issue=4,
              ypp_bufs=4, skip_conn_dma=False, skip_mm=False):
    """Build the per-core kernel.

    repeat: re-run the whole body N times (benchmark-only; deliverable uses 1)
    B: K-chunks per DMA batch (one descriptor run = B*AS*elt bytes/partition)
    bufs: conn stream-pool buffering depth
    conn_dt: dtype of the conn stream ("f16"/"bf16"/"f32r")
    skip_conn_dma/skip_mm: timing ablations (results become garbage)
    """
    CD, _ = _conn_dtype(conn_dt)
    nc = bacc.Bacc("TRN2", target_bir_lowering=False, debug=False)

    # partition-major conn: [p, kc*AS + a] = conn2dT[kc*128+p, a]
    conn_pm = nc.dram_tensor("conn_pm", [KP, KC * AS], CD, kind="ExternalInput").ap()
    # bond_t [24, AS] and bf2 [24, O] packed side by side -> one DMA
    bpack = nc.dram_tensor("bpack", [KB, AS + OUT_DEPTH], F32, kind="ExternalInput").ap()
    # xT [64, A] and Wr [64, F*O] packed side by side -> one DMA
    xw = nc.dram_tensor("xw", [IN_DEPTH, A + F * OUT_DEPTH], F32,
                        kind="ExternalInput").ap()
    out_t = nc.dram_tensor("out_t", [OUT_DEPTH, AS], F32, kind="ExternalOutput").ap()

    # conn DMA batch sizes: big batches for bandwidth, tapered tail so the
    # final accumulating matmuls (which gate the output copy) start early
    batches = [B] * (KC // B - 1) + [B // 2, B // 4, B // 8, B // 8]
    assert sum(batches) == KC
    NG = KC // YG               # y groups total

    with tile.TileContext(nc) as tc:
        with (
            tc.tile_pool(name="const", bufs=1) as cpool,
            tc.tile_pool(name="ypool", bufs=y_ring) as ypool,
            tc.tile_pool(name="stream", bufs=bufs) as spool,
            tc.tile_pool(name="psum", bufs=2, space="PSUM") as ppool,
            tc.tile_pool(name="ypsum", bufs=ypp_bufs, space="PSUM") as ypp,
        ):
            # small input DMAs, packed, on the second HWDGE ring (ACT) so the
            # conn stream owns the SP ring from t=0
            bp_sb = cpool.tile([KB, AS + OUT_DEPTH], F32)
            nc.scalar.dma_start(bp_sb[:], bpack[:])
            bond_sb = bp_sb[:, :AS]
            bf2_sb = bp_sb[:, AS:AS + OUT_DEPTH]
            xw_sb = cpool.tile([IN_DEPTH, A + F * OUT_DEPTH], F32)
            nc.scalar.dma_start(xw_sb[:], xw[:])
            xt_sb = xw_sb[:, :A]
            wr_sb = xw_sb[:, A:A + F * OUT_DEPTH]

            for rep in range(repeat):
                ygroups = {}

                def y_chunk_ap(kc):
                    g, j = divmod(kc, YG)
                    return ygroups[g][:, j * OUT_DEPTH:(j + 1) * OUT_DEPTH]

                def y_group(g):
                    # Y[kc=(f,nb)] chunk = xT[:, nb-block].T @ Wr[:, f-block];
                    # each chunk is consumed by exactly one conn matmul, so
                    # groups live in a small ring (ypool bufs) not a flat 6.3MB
                    yps = ypp.tile([KP, YG * OUT_DEPTH], F32, tag="yps")
                    for j in range(YG):
                        kc = g * YG + j
                        f, nb = divmod(kc, NBLK)
                        nc.tensor.matmul(
                            yps[:, j * OUT_DEPTH:(j + 1) * OUT_DEPTH],
                            xt_sb[:, nb * KP:(nb + 1) * KP],
                            wr_sb[:, f * OUT_DEPTH:(f + 1) * OUT_DEPTH],
                            start=(j == 0),
                            stop=(j == YG - 1),
                        )
                    yt = ypool.tile([KP, YG * OUT_DEPTH], CD, tag="y",
                                    name=f"yt_{rep}_{g}")
                    if CD == mybir.dt.float32r:
                        nc.vector.tensor_copy(yt[:], yps[:].bitcast(CD))
                    else:
                        nc.vector.tensor_copy(yt[:], yps[:])
                    ygroups[g] = yt

                # issue the first conn batch DMAs before anything else so
                # the SP ring streams from t=0
                ctiles = {}
                starts = []
                k0 = 0
                for bsz in batches:
                    starts.append(k0)
                    k0 += bsz

                def issue_conn(bt):
                    bsz = batches[bt]
                    ctile = spool.tile([KP, bsz * AS], CD, tag="conn",
                                       name=f"conn_{rep}_{bt}")
                    if skip_conn_dma:
                        nc.sync.dma_start(ctile[:, :8], conn_pm[:, :8])
                    else:
                        nc.sync.dma_start(
                            ctile[:],
                            conn_pm[:, starts[bt] * AS:(starts[bt] + bsz) * AS])
                    ctiles[bt] = ctile

                for bt in range(min(pre_issue, len(batches))):
                    issue_conn(bt)

                # all Y groups upfront: PE does this under the conn DMA
                # burst, so the 192 conn matmuls then run back-to-back with
                # no PE<->DVE ping-pong stalls between batches
                for g in range(NG):
                    y_group(g)

                acc = ppool.tile([OUT_DEPTH, AS], F32, tag="acc")

                # bond term opens the PSUM accumulation group
                nc.tensor.matmul(acc[:], bf2_sb[:], bond_sb[:], start=True, stop=False)

                for bt, bsz in enumerate(batches):
                    for b in range(bsz):
                        kc = starts[bt] + b
                        if skip_mm and not (kc == KC - 1 or kc % YG == 0):
                            continue
                        nc.tensor.matmul(
                            acc[:],
                            y_chunk_ap(kc),
                            ctiles[bt][:, b * AS:(b + 1) * AS],
                            start=False,
                            stop=(kc == KC - 1),
                        )
                    # prefetch next batch's DMA
                    nxt = bt + pre_issue
                    if nxt < len(batches):
                        issue_conn(nxt)

                out_sb = spool.tile([OUT_DEPTH, AS], F32, tag="osb")
                nc.vector.tensor_copy(out_sb[:], acc[:])
                nc.sync.dma_start(out_t[:], out_sb[:])

    nc.compile()
    return nc


def _prep(node_property_tensor, connectivity_tensor, bond_property_tensor,
          property_filters, bond_filters, conn_dt="f16"):
    x = np.asarray(node_property_tensor, dtype=np.float32)
    conn = np.asarray(connectivity_tensor, dtype=np.float32)
    bp = np.asarray(bond_property_tensor, dtype=np.float32)
    pf = np.asarray(property_filters, dtype=np.float32)
    bf = np.asarray(bond_filters, dtype=np.float32)
    _, npdt = _conn_dtype(conn_dt)

    W = pf * bf[:, :, 0:1]                                # (O, F, I)
    wr = np.ascontiguousarray(W.transpose(2, 1, 0).reshape(IN_DEPTH, F * OUT_DEPTH))
    bf2 = np.ascontiguousarray(bf[:, :, 1:3].reshape(OUT_DEPTH, KB).T)  # (24, O)
    xw = np.ascontiguousarray(np.concatenate([x.T, wr], axis=1))  # (64, A + F*O)

    # partition-major conn per core: pm[c, p, (f*NBLK+nb)*AS + a]
    #   = conn[c*AS+a, nb*KP+p, f]   (k = f*A + n, kc = f*NBLK + nb)
    cv = conn.reshape(NCORES, AS, NBLK, KP, F)            # [c, a, nb, p, f]
    pm = cv.transpose(0, 3, 4, 2, 1).astype(npdt)         # [c, p, f, nb, a]
    pm = pm.reshape(NCORES, KP, KC * AS)

    in_maps = []
    for c in range(NCORES):
        sl = slice(c * AS, (c + 1) * AS)
        bond_tc = bp[sl].reshape(AS, KB).T                # (24, AS)
        in_maps.append({
            "conn_pm": pm[c],
            "bpack": np.ascontiguousarray(
                np.concatenate([bond_tc, bf2], axis=1)),  # (24, AS + O)
            "xw": xw,
        })
    return in_maps


def kernel(node_property_tensor, connectivity_tensor, bond_property_tensor,
           property_filters, bond_filters):
    in_maps = _prep(node_property_tensor, connectivity_tensor,
                    bond_property_tensor, property_filters, bond_filters)

    if "nc" not in _cache:
        _cache["nc"] = _build_nc()
    nc = _cache["nc"]

    res = run_bass_kernel_spmd(nc, in_maps, core_ids=list(range(NCORES)))

    out = np.empty((A, OUT_DEPTH), dtype=np.float32)
    for c in range(NCORES):
        out[c * AS:(c + 1) * AS, :] = res.results[c]["out_t"].T
    return out
